# revision 18
# baseline (speedup 1.0000x reference)
"""NT-Xent (SimCLR) contrastive loss on 8 Trainium2 NeuronCores.

Math (reference):
    z = l2_normalize(concat(emb_i, emb_j))          # [2N, d]
    logits = (z @ z.T) / T,  T = 0.5
    denom_i = sum_j exp(logits[i,j]) - exp(logits[i,i])
    pos_i   = logits[i, (i+N) mod 2N]
    loss = mean(log(denom_i) - pos_i)

Sharding: row-parallel over the 2N=8192 rows, 1024 rows per core. Each core
receives the full embedding matrix ROTATED so its own row block comes first;
rotation keeps the positive-pair offset at +4096 and leaves row sums (the
denominators) unchanged, so one static SPMD program serves all 8 cores.

Per-core device program:
  - DVE: per-row sum-of-squares (fused square+reduce), normalize+cast to bf16
  - ACT: inv_norm = exp(-0.5*ln(ss))  (Ln/Exp share one ACT table set)
  - DMA: normalized bf16 W round-trips through DRAM for two big xbar
    transposes -> W^T stored as two [128, 2N] halves (d on partitions)
  - PE : sim row-block = W_my^T.T @ W^T in [128,512] PSUM tiles, K=256
  - ACT: in-place exp(2*sim) over [128, COLCHUNK] PSUM tiles with accum_out
    producing the row sums; the exp matrix itself is never stored
  - tail: pos/diag dot products on DVE (tensor_tensor_reduce), Ln, and
    per-row loss terms DMA'd out as [128, MYT] f32
Host sums the 8 partial term blocks and divides by 2N.
"""

import sys

if "/opt/trn_rl_repo" not in sys.path:
    sys.path.insert(0, "/opt/trn_rl_repo")

from contextlib import ExitStack

import numpy as np

import functools

import concourse.bacc as bacc
import concourse.bass as bass
import concourse.mybir as mybir
import concourse.tile as tile
from concourse.bass_utils import run_bass_kernel_spmd

# Make Exp/Ln resolve only to their shared ACT table set so bacc's
# insert_act_table_loads emits one LoadActFuncSet instead of bouncing
# between exp_and_others and natural_log (2.7us per switch, 17 switches).
# Set names and order are preserved, so act_func_set_ids stay aligned
# with act_info.json.
_orig_get_activation_tables = bacc.get_activation_tables


@functools.cache
def _patched_get_activation_tables(module_arch):
    tables = _orig_get_activation_tables(module_arch)
    combined = "natural_log_exp_and_others"
    if combined not in tables:
        return tables
    exp_ln = {
        mybir.ActivationFunctionType.Exp,
        mybir.ActivationFunctionType.Ln,
    }
    return {
        name: (set(fns) if name == combined else set(fns) - exp_ln)
        for name, fns in tables.items()
    }


bacc.get_activation_tables = _patched_get_activation_tables

N = 4096
D = 256
TWO_N = 2 * N
NCORES = 8
RPC = TWO_N // NCORES  # 1024 rows per core

FP32 = mybir.dt.float32
BF16 = mybir.dt.bfloat16
AF = mybir.ActivationFunctionType
ALU = mybir.AluOpType


def build_nc(two_n=TWO_N, d=D, rpc=RPC, inplace_exp=True, parts="ABC", reps=1):
    assert d == 256, "kernel assumes d=256 (two K=128 halves)"
    nt = two_n // 128          # total 128-row tiles
    myt = rpc // 128           # my row tiles
    tpg = min(8, nt)           # tiles per load/store/transpose group
    ngroups = nt // tpg
    colchunk = min(2048, two_n)  # psum tile free size
    # narrow leading column chunks so the first matmuls only need the first
    # transposed half-group (shrinks the pipeline-fill head)
    chunks = [(0, 512), (512, 512)] if colchunk >= 1024 else []
    c = 1024 if chunks else 0
    if c < two_n and (two_n - c) % colchunk:
        chunks.append((c, colchunk // 2))
        c += colchunk // 2
    while c < two_n:
        chunks.append((c, colchunk))
        c += colchunk
    nq = len(chunks)
    pair_off_t = nt // 2         # pair block tile offset

    nc = bacc.Bacc("TRN2", target_bir_lowering=False, debug=False)
    embs = nc.dram_tensor("embs", [two_n, d], FP32, kind="ExternalInput")
    out_terms = nc.dram_tensor("loss_terms", [128, myt], FP32, kind="ExternalOutput")

    with ExitStack() as ctx:
        tc = ctx.enter_context(tile.TileContext(nc))

        raw_pool = ctx.enter_context(tc.tile_pool(name="raw", bufs=ngroups))
        wnat_pool = ctx.enter_context(tc.tile_pool(name="wnat", bufs=ngroups))
        wt_pool = ctx.enter_context(tc.tile_pool(name="wt", bufs=1))
        small = ctx.enter_context(tc.tile_pool(name="small", bufs=1))
        scratch = ctx.enter_context(tc.tile_pool(name="scratch", bufs=2))
        dram_pool = ctx.enter_context(tc.tile_pool(name="dram", bufs=1, space="DRAM"))
        psum_pool = ctx.enter_context(tc.tile_pool(name="psum", bufs=2, space="PSUM"))

        wdram = dram_pool.tile([two_n, d], BF16)
        # W^T halves: wt[h][p, j] = w[j, 128*h + p]
        wt0 = wt_pool.tile([128, two_n], BF16, tag="wt0")
        wt1 = wt_pool.tile([128, two_n], BF16, tag="wt1")

        nsq = small.tile([128, nt], FP32)   # per-row sum of squares
        lns = small.tile([128, nt], FP32)   # ln(nsq)
        inv = small.tile([128, nt], FP32)   # nsq ** -0.5
        rowsums = small.tile([128, myt, nq], FP32)

        embs_v = embs[:].rearrange("(g t p) d -> g p t d", t=tpg, p=128)
        wdram_v = wdram[:].rearrange("(g t p) d -> g p t d", t=tpg, p=128)

        wnat = []
        raws = []
        # issue every load up front on the gpsimd (SWDGE) ring, casting
        # fp32 -> bf16 in the DMA so the SP ring only carries stores and
        # transposes and later groups' loads are never blocked by
        # transpose waits
        for g in range(ngroups if "A" in parts else 0):
            raw = raw_pool.tile([128, tpg, d], BF16, tag="raw")
            raws.append(raw)
            nc.gpsimd.dma_start(out=raw[:], in_=embs_v[g])
        for g in range(ngroups if "A" in parts else 0):
            raw = raws[g]

            # per-row sum of squares for the tiles in this group
            for t in range(tpg):
                k = g * tpg + t
                sq = scratch.tile([128, d], BF16, tag="sq")
                nc.vector.scalar_tensor_tensor(
                    out=sq[:],
                    in0=raw[:, t, :],
                    scalar=0.0,
                    in1=raw[:, t, :],
                    op0=ALU.bypass,
                    op1=ALU.mult,
                    accum_out=nsq[:, k : k + 1],
                )

            gs = slice(g * tpg, (g + 1) * tpg)
            # inv = nsq^-0.5 via exp(-0.5*ln(nsq)): keeps ACT in one table set
            nc.scalar.activation(out=lns[:, gs], in_=nsq[:, gs], func=AF.Ln)
            nc.scalar.activation(out=inv[:, gs], in_=lns[:, gs], func=AF.Exp, scale=-0.5)

            wn = wnat_pool.tile([128, tpg, d], BF16, tag="wnat")
            wnat.append(wn)
            for t in range(tpg):
                k = g * tpg + t
                nc.vector.tensor_scalar_mul(wn[:, t, :], raw[:, t, :], inv[:, k : k + 1])

            # bounce through DRAM so the transpose runs as one big xbar DMA.
            # group 0 goes at half-group granularity: its first 512 rows are
            # what the first matmuls wait on.
            nhalf = 2 if (g == 0 and tpg >= 2) else 1
            th = tpg // nhalf
            for h in range(nhalf):
                nc.sync.dma_start(
                    out=wdram_v[g][:, h * th : (h + 1) * th, :],
                    in_=wn[:, h * th : (h + 1) * th, :],
                )
                r0 = (g * tpg + h * th) * 128
                rows = th * 128
                nc.sync.dma_start_transpose(
                    out=wt0[:, r0 : r0 + rows], in_=wdram[r0 : r0 + rows, 0:128]
                )
                nc.sync.dma_start_transpose(
                    out=wt1[:, r0 : r0 + rows], in_=wdram[r0 : r0 + rows, 128:256]
                )

        if "B" not in parts or "C" not in parts:
            nc.vector.memset(rowsums[:], 1.0)
        # main loop: sim tiles for my rows x all columns
        # (reps>1 repeats the main loop for wall-clock benchmarking)
        import itertools

        for _rep, q in itertools.product(
            range(reps), range(nq if ("B" in parts and "A" in parts) else 0)
        ):
            cbase, cw = chunks[q]
            for i in range(myt):
                ps = psum_pool.tile([128, cw], FP32, tag="ps")
                for s in range(cw // 512):  # 512-wide matmuls (one PSUM bank each)
                    c0 = cbase + s * 512
                    sub = ps[:, s * 512 : (s + 1) * 512]
                    nc.tensor.matmul(
                        sub,
                        lhsT=wt0[:, i * 128 : (i + 1) * 128],
                        rhs=wt0[:, c0 : c0 + 512],
                        start=True,
                        stop=False,
                    )
                    nc.tensor.matmul(
                        sub,
                        lhsT=wt1[:, i * 128 : (i + 1) * 128],
                        rhs=wt1[:, c0 : c0 + 512],
                        start=False,
                        stop=True,
                    )
                # exp(2*sim); only the row sum survives
                if inplace_exp:
                    eout = ps[:]
                else:
                    eout_t = scratch.tile([128, colchunk], BF16, tag="eout")
                    eout = eout_t[:]
                nc.scalar.activation(
                    out=eout,
                    in_=ps[:],
                    func=AF.Exp,
                    scale=2.0,
                    accum_out=rowsums[:, i, q : q + 1],
                )

        # tail: positives, diagonal correction, loss terms
        posv = small.tile([128, myt], FP32)
        ssqv = small.tile([128, myt], FP32)
        if "C" not in parts or "A" not in parts:
            nc.vector.memset(posv[:], 0.25)
            nc.vector.memset(ssqv[:], 1.0)
        for t in range(myt if ("C" in parts and "A" in parts) else 0):
            tp = t + pair_off_t
            w_my = wnat[t // tpg][:, t % tpg, :]
            w_pair = wnat[tp // tpg][:, tp % tpg, :]
            tsc = scratch.tile([128, d], BF16, tag="tsc")
            nc.vector.scalar_tensor_tensor(
                out=tsc[:],
                in0=w_my,
                scalar=0.0,
                in1=w_pair,
                op0=ALU.bypass,
                op1=ALU.mult,
                accum_out=posv[:, t : t + 1],
            )
            tsc2 = scratch.tile([128, d], BF16, tag="tsc2")
            nc.vector.scalar_tensor_tensor(
                out=tsc2[:],
                in0=w_my,
                scalar=0.0,
                in1=w_my,
                op0=ALU.bypass,
                op1=ALU.mult,
                accum_out=ssqv[:, t : t + 1],
            )

        diagexp = small.tile([128, myt], FP32)
        nc.scalar.activation(out=diagexp[:], in_=ssqv[:], func=AF.Exp, scale=2.0)

        den = small.tile([128, myt], FP32)
        if nq > 1:
            nc.vector.tensor_reduce(
                out=den[:], in_=rowsums[:], axis=mybir.AxisListType.X, op=ALU.add
            )
        else:
            nc.vector.tensor_copy(den[:], rowsums[:, :, 0])
        den2 = small.tile([128, myt], FP32)
        nc.vector.tensor_sub(den2[:], den[:], diagexp[:])
        logden = small.tile([128, myt], FP32)
        nc.scalar.activation(out=logden[:], in_=den2[:], func=AF.Ln)

        # loss_term = log(denom) - 2 * (w_my . w_pair)
        lossT = small.tile([128, myt], FP32)
        nc.vector.scalar_tensor_tensor(
            out=lossT[:],
            in0=posv[:],
            scalar=-2.0,
            in1=logden[:],
            op0=ALU.mult,
            op1=ALU.add,
        )
        nc.gpsimd.dma_start(out=out_terms[:], in_=lossT[:])

    nc.finalize()
    return nc


_NC_CACHE = {}


def _get_nc():
    if "nc" not in _NC_CACHE:
        _NC_CACHE["nc"] = build_nc()
    return _NC_CACHE["nc"]


def _make_in_maps(emb_i, emb_j):
    allA = np.concatenate(
        [np.asarray(emb_i, np.float32), np.asarray(emb_j, np.float32)], axis=0
    )
    in_maps = []
    for c in range(NCORES):
        rot = np.roll(allA, -c * RPC, axis=0)
        in_maps.append({"embs": np.ascontiguousarray(rot)})
    return in_maps


def run_device(emb_i, emb_j, **run_kwargs):
    nc = _get_nc()
    in_maps = _make_in_maps(emb_i, emb_j)
    return run_bass_kernel_spmd(nc, in_maps, core_ids=list(range(NCORES)), **run_kwargs)


def kernel(emb_i, emb_j):
    res = run_device(emb_i, emb_j)
    total = 0.0
    for r in res.results:
        total += r["loss_terms"].astype(np.float64).sum()
    return np.array(total / float(TWO_N), dtype=np.float32)


if __name__ == "__main__":
    rng = np.random.default_rng(0)
    ei = rng.standard_normal((N, D)).astype(np.float32)
    ej = rng.standard_normal((N, D)).astype(np.float32)
    print(kernel(ei, ej))
